# revision 8
# baseline (speedup 1.0000x reference)
"""Trainium2 Bass kernel for the 5-head detection tower (nn_DFD_10849087390476).

Network (per head h of 5): 1x1-conv tower on x [B,64,H,W]:
    h1 = relu(Win x + bin)
    h2 = h1 + relu(Wh0 h1 + bh0)
    h3 = h2 + relu(Wh1 h2 + bh1)
    out_h = Wout h3 + bout
Output = concat over heads: channels [cls 81, obj 2, box 4, pos 64, ins 128] = 279.

Sharding: data-parallel over (batch, H/2) -> 8 shards of 32768 pixels.
Per core pixels split in two 16384-px halves: A-px channels on SBUF partitions
0-63, B-px on 64-127 (2 pixels per column). Heads paired into three sections
with block-diagonal stationaries: co=cls+obj, bp=box+pos, ins=ins(A/B).

v2 layout: stage-major over groups of G=4 pair-tiles (512 cols each) so every
stationary streams 2048+ columns back-to-back (PE p-state stays at 2.4 GHz).
Activations in bf16: hidden residual adds run on Pool/DVE-2x from SBUF,
layer-2 residual is a fused scalar_tensor_tensor, in/l1 relus on the scalar
engine, out-proj copy (+bias via per-partition scalar AP) split scalar/DVE.
Output staged and stored as bf16 (halves write traffic), upcast on host.
"""
import numpy as np
import ml_dtypes

from concourse import bacc, tile
import concourse.mybir as mybir
from concourse.bass_utils import run_bass_kernel_spmd

F32 = mybir.dt.float32
F32R = mybir.dt.float32r
BF16 = mybir.dt.bfloat16
AF = mybir.ActivationFunctionType
ALU = mybir.AluOpType

B, C, H, W = 4, 64, 256, 256
NCORES = 8
NPX = (B * H * W) // NCORES          # 32768 pixels per core
NG = NPX // 2                        # 16384 per A/B half
T = 512                              # columns (pixel pairs) per matmul tile
GT = 4                               # pair-tiles per stage group
NGRP = NG // (GT * T)                # 8 groups per core
OD = 279

SECS = ("co", "bp", "ins")
MO = {"co": 83, "bp": 68, "ins": 128}
OCH = {"co": (0, 83), "bp": (83, 151), "ins": (151, 279)}

# ---- packed fp32 weight tensor (in-proj stationaries, bitcast f32r) ----
_WF = {}
_c = 0
for _n in ("sin_co", "sin_bp", "sin_ins"):
    _WF[_n] = (_c, 128); _c += 128
WF_COLS = _c

# ---- packed bf16 weight tensor (l1/l2/out stationaries) ----
_WB = {}
_c = 0
for _n in ("sl1_co", "sl1_bp", "sl1_ins", "sl2_co", "sl2_bp", "sl2_ins"):
    _WB[_n] = (_c, 128); _c += 128
for _s in SECS:
    _WB["sout_" + _s] = (_c, MO[_s] if _s != "ins" else 128); _c += _WB["sout_" + _s][1]
WB_COLS = _c

# ---- packed fp32 bias tensor (one column each) ----
_BI = {}
_c = 0
for _n in ("bin_co", "bin_bp", "bin_ins", "bh1_co", "bh1_bp", "bh1_ins",
           "bh2_co", "bh2_bp", "bh2_ins", "bout_co", "bout_bp", "bout_ins"):
    _BI[_n] = _c; _c += 1
BI_COLS = _c

_last_results = None                 # test.py reads exec_time_ns from here
_cache = {}


def _bd(a, b):
    out = np.zeros((a.shape[0] + b.shape[0], a.shape[1] + b.shape[1]), np.float32)
    out[:a.shape[0], :a.shape[1]] = a
    out[a.shape[0]:, a.shape[1]:] = b
    return out


def _build(fast: bool):
    nc = bacc.Bacc("TRN2", target_bir_lowering=False, debug=False)

    xs_d = nc.dram_tensor("xs", [128, NG], F32, kind="ExternalInput")
    wf_d = nc.dram_tensor("wf", [128, WF_COLS], F32, kind="ExternalInput")
    wb_d = nc.dram_tensor("wb", [128, WB_COLS], BF16, kind="ExternalInput")
    bi_d = nc.dram_tensor("bi", [128, BI_COLS], F32, kind="ExternalInput")
    out_d = nc.dram_tensor("out", [OD, NPX], BF16, kind="ExternalOutput")

    GC = GT * T                      # columns per group (2048)

    with tile.TileContext(nc) as tc:
        with tc.tile_pool(name="const", bufs=1) as cpool, \
             tc.tile_pool(name="xp", bufs=2) as xpool, \
             tc.tile_pool(name="hp", bufs=5) as hpool, \
             tc.tile_pool(name="op", bufs=2) as opool, \
             tc.tile_pool(name="psb", bufs=3, space="PSUM") as psb, \
             tc.tile_pool(name="pss", bufs=2, space="PSUM") as pss:

            wf_t = cpool.tile([128, WF_COLS], F32R, tag="wf")
            nc.sync.dma_start(out=wf_t[:], in_=wf_d.ap().bitcast(F32R))
            wb_t = cpool.tile([128, WB_COLS], BF16, tag="wb")
            nc.sync.dma_start(out=wb_t[:], in_=wb_d.ap())
            bi_t = cpool.tile([128, BI_COLS], F32, tag="bi")
            nc.sync.dma_start(out=bi_t[:], in_=bi_d.ap())

            def wfa(name):
                c0, n = _WF[name]
                return wf_t[:, c0:c0 + n]

            def wba(name):
                c0, n = _WB[name]
                return wb_t[:, c0:c0 + n]

            def bia(name, rows=128):
                c0 = _BI[name]
                return bi_t[0:rows, c0:c0 + 1]

            def load_x(g):
                x_t = xpool.tile([128, GC], F32R, tag="x", name=f"x_{g}")
                nc.sync.dma_start(
                    out=x_t[:],
                    in_=xs_d.ap()[:, g * GC:(g + 1) * GC].bitcast(F32R))
                return x_t

            x_next = load_x(0)
            for g in range(NGRP):
                x_t = x_next
                if g + 1 < NGRP:
                    x_next = load_x(g + 1)

                # output staging for this group: A and B halves per section
                oA = {s: opool.tile([MO[s], GC], BF16, tag=f"oA{s}", name=f"oA_{s}{g}")
                      for s in SECS}
                oB = {s: opool.tile([MO[s], GC], BF16, tag=f"oB{s}", name=f"oB_{s}{g}")
                      for s in SECS}

                for s in SECS:
                    wide = 2 * T if s != "ins" else T    # cols per pair-tile
                    pstag = "big" if s != "ins" else "small"
                    pspool = psb if s != "ins" else pss

                    # ---- in-proj ----
                    ps_in = []
                    for t in range(GT):
                        xg = x_t[:, t * T:(t + 1) * T]
                        p = pspool.tile([128, wide], F32, tag=pstag, name=f"pin_{s}{t}")
                        if s == "ins":
                            nc.tensor.matmul(p[:], wfa("sin_ins"), xg,
                                             start=True, stop=True)
                        else:
                            nc.tensor.matmul(p[:, 0:T], wfa("sin_" + s)[0:64, :],
                                             xg[0:64, :], start=True, stop=True)
                            nc.tensor.matmul(p[:, T:2 * T],
                                             wfa("sin_" + s)[64:128, :],
                                             xg[64:128, :], start=True, stop=True)
                        ps_in.append(p)
                    h1 = []
                    for t in range(GT):
                        ht = hpool.tile([128, wide], BF16, tag=f"h1{s}", name=f"h1_{s}{t}")
                        nc.scalar.activation(ht[:], ps_in[t][:], AF.Relu,
                                             bias=bia("bin_" + s), scale=1.0)
                        h1.append(ht)

                    # ---- hidden layer 1: r1 = relu(W1 h1 + bh1); h2 = h1 + r1
                    ps_l1 = []
                    for t in range(GT):
                        p = pspool.tile([128, wide], F32, tag=pstag, name=f"pl1_{s}{t}")
                        for k in range(wide // T):
                            nc.tensor.matmul(p[:, k * T:(k + 1) * T], wba(f"sl1_{s}"),
                                             h1[t][:, k * T:(k + 1) * T],
                                             start=True, stop=True)
                        ps_l1.append(p)
                    h2 = []
                    for t in range(GT):
                        rt = hpool.tile([128, wide], BF16, tag=f"r1{s}", name=f"r1_{s}{t}")
                        nc.scalar.activation(rt[:], ps_l1[t][:], AF.Relu,
                                             bias=bia("bh1_" + s), scale=1.0)
                        ht = hpool.tile([128, wide], BF16, tag=f"h2{s}", name=f"h2_{s}{t}")
                        # bf16 SBUF adds: Pool for co/bp, DVE (2x mode) for ins
                        if s == "ins":
                            nc.vector.tensor_tensor(ht[:], h1[t][:], rt[:], ALU.add)
                        else:
                            nc.gpsimd.tensor_tensor(ht[:], h1[t][:], rt[:], ALU.add)
                        h2.append(ht)

                    # ---- hidden layer 2 (fused residual on DVE) ----
                    ps_l2 = []
                    for t in range(GT):
                        p = pspool.tile([128, wide], F32, tag=pstag, name=f"pl2_{s}{t}")
                        for k in range(wide // T):
                            nc.tensor.matmul(p[:, k * T:(k + 1) * T], wba(f"sl2_{s}"),
                                             h2[t][:, k * T:(k + 1) * T],
                                             start=True, stop=True)
                        ps_l2.append(p)
                    h3 = []
                    for t in range(GT):
                        ht = hpool.tile([128, wide], BF16, tag=f"h3{s}", name=f"h3_{s}{t}")
                        if fast:
                            # h3 = max(psum, 0) + h2
                            nc.vector.scalar_tensor_tensor(
                                ht[:], ps_l2[t][:], 0.0, h2[t][:], ALU.max, ALU.add)
                        else:
                            rt = hpool.tile([128, wide], BF16, tag=f"r2{s}",
                                            name=f"r2_{s}{t}")
                            nc.scalar.activation(rt[:], ps_l2[t][:], AF.Relu,
                                                 bias=bia("bh2_" + s), scale=1.0)
                            nc.vector.tensor_tensor(ht[:], h2[t][:], rt[:], ALU.add)
                        h3.append(ht)

                    # ---- out-proj ----
                    mo = MO[s]
                    for t in range(GT):
                        p = psb.tile([128, 2 * T], F32, tag="big", name=f"po_{s}{t}")
                        if s == "ins":
                            # A-px into cols 0:T, B-px into T:2T
                            nc.tensor.matmul(p[:, 0:T], wba("sout_ins")[0:64, :],
                                             h3[t][0:64, :], start=True, stop=True)
                            nc.tensor.matmul(p[:, T:2 * T],
                                             wba("sout_ins")[64:128, :],
                                             h3[t][64:128, :], start=True, stop=True)
                        else:
                            nc.tensor.matmul(p[0:mo, 0:T], wba("sout_" + s),
                                             h3[t][:, 0:T], start=True, stop=True)
                            nc.tensor.matmul(p[0:mo, T:2 * T], wba("sout_" + s),
                                             h3[t][:, T:2 * T], start=True, stop=True)
                        # copy psum -> staging with bias; scalar takes co,
                        # DVE takes bp and ins
                        dA = oA[s][:, t * T:(t + 1) * T]
                        dB = oB[s][:, t * T:(t + 1) * T]
                        if s == "co":
                            nc.scalar.activation(dA, p[0:mo, 0:T], AF.Identity,
                                                 bias=bia("bout_co", rows=mo), scale=1.0)
                            nc.scalar.activation(dB, p[0:mo, T:2 * T], AF.Identity,
                                                 bias=bia("bout_co", rows=mo), scale=1.0)
                        else:
                            bap = bia("bout_" + s, rows=mo)
                            nc.vector.tensor_scalar(dA, p[0:mo, 0:T], bap, None, ALU.add)
                            nc.vector.tensor_scalar(dB, p[0:mo, T:2 * T], bap, None, ALU.add)

                # ---- flush group staging (pool engine: cheap DMA issue) ----
                base = g * GC
                for s in SECS:
                    lo, hi = OCH[s]
                    nc.gpsimd.dma_start(out=out_d.ap()[lo:hi, base:base + GC],
                                        in_=oA[s][:])
                    nc.gpsimd.dma_start(out=out_d.ap()[lo:hi, NG + base:NG + base + GC],
                                        in_=oB[s][:])

    nc.compile()
    return nc


def _prep_inputs(inputs):
    f32 = np.float32
    bf16 = ml_dtypes.bfloat16

    def wT(name):
        return np.ascontiguousarray(np.asarray(inputs[name], f32).T)

    # fp32 (f32r) in-proj stationaries: rows 0:64 serve A-px, 64:128 B-px
    wfm = {}
    wfm["sin_co"] = np.concatenate(
        [np.concatenate([wT("cls_Win"), wT("obj_Win")], 1)] * 2, 0)
    wfm["sin_bp"] = np.concatenate(
        [np.concatenate([wT("box_Win"), wT("pos_Win")], 1)] * 2, 0)
    wfm["sin_ins"] = _bd(wT("ins_Win"), wT("ins_Win"))
    wf = np.zeros((128, WF_COLS), f32)
    for name, (c0, n) in _WF.items():
        wf[:, c0:c0 + n] = wfm[name]

    # bf16 stationaries for l1/l2/out
    wbm = {}
    for l in (1, 2):
        wbm[f"sl{l}_co"] = _bd(np.asarray(inputs["cls_Wh"][l - 1], f32).T,
                               np.asarray(inputs["obj_Wh"][l - 1], f32).T)
        wbm[f"sl{l}_bp"] = _bd(np.asarray(inputs["box_Wh"][l - 1], f32).T,
                               np.asarray(inputs["pos_Wh"][l - 1], f32).T)
        wbm[f"sl{l}_ins"] = _bd(np.asarray(inputs["ins_Wh"][l - 1], f32).T,
                                np.asarray(inputs["ins_Wh"][l - 1], f32).T)
    wbm["sout_co"] = _bd(wT("cls_Wout"), wT("obj_Wout"))        # [128, 83]
    wbm["sout_bp"] = _bd(wT("box_Wout"), wT("pos_Wout"))        # [128, 68]
    wbm["sout_ins"] = np.concatenate([wT("ins_Wout")] * 2, 0)   # [128, 128]
    wb = np.zeros((128, WB_COLS), f32)
    for name, (c0, n) in _WB.items():
        wb[:, c0:c0 + n] = wbm[name]
    wb = wb.astype(bf16)

    def colv(v):
        return np.asarray(v, f32).reshape(-1)

    bim = {}
    bim["bin_co"] = np.concatenate([colv(inputs["cls_bin"]), colv(inputs["obj_bin"])])
    bim["bin_bp"] = np.concatenate([colv(inputs["box_bin"]), colv(inputs["pos_bin"])])
    bim["bin_ins"] = np.concatenate([colv(inputs["ins_bin"])] * 2)
    for l in (1, 2):
        bim[f"bh{l}_co"] = np.concatenate([colv(inputs["cls_bh"][l - 1]),
                                           colv(inputs["obj_bh"][l - 1])])
        bim[f"bh{l}_bp"] = np.concatenate([colv(inputs["box_bh"][l - 1]),
                                           colv(inputs["pos_bh"][l - 1])])
        bim[f"bh{l}_ins"] = np.concatenate([colv(inputs["ins_bh"][l - 1])] * 2)
    bim["bout_co"] = np.concatenate([colv(inputs["cls_bout"]), colv(inputs["obj_bout"])])
    bim["bout_bp"] = np.concatenate([colv(inputs["box_bout"]), colv(inputs["pos_bout"])])
    bim["bout_ins"] = colv(inputs["ins_bout"])
    bi = np.zeros((128, BI_COLS), f32)
    for name, c0 in _BI.items():
        v = bim[name]
        bi[:v.shape[0], c0] = v

    # fast path requires zero layer-2 hidden biases (fused STT cannot apply a
    # bias before the relu)
    fast = not any(np.any(np.asarray(inputs[k + "_bh"][1])) for k in
                   ("cls", "obj", "box", "pos", "ins"))

    x = np.asarray(inputs["x"], f32)
    in_maps = []
    for c in range(NCORES):
        b, hh = c // 2, c % 2
        xs = x[b, :, hh * 128:(hh + 1) * 128, :].reshape(64, NPX)
        xsr = np.ascontiguousarray(
            np.concatenate([xs[:, :NG], xs[:, NG:]], axis=0))   # [128, NG]
        in_maps.append({"xs": xsr, "wf": wf, "wb": wb, "bi": bi})
    return in_maps, fast


def kernel(**inputs) -> np.ndarray:
    global _last_results
    in_maps, fast = _prep_inputs(inputs)
    if fast not in _cache:
        _cache[fast] = _build(fast)
    nc = _cache[fast]
    res = run_bass_kernel_spmd(nc, in_maps, core_ids=list(range(NCORES)))
    _last_results = res

    out = np.empty((B, OD, H, W), np.float32)
    for c in range(NCORES):
        b, hh = c // 2, c % 2
        o = np.asarray(res.results[c]["out"])
        if o.dtype != np.float32:
            o = o.astype(np.float32)
        out[b, :, hh * 128:(hh + 1) * 128, :] = o.reshape(OD, 128, W)
    return out


# revision 10
# speedup vs baseline: 1.1858x; 1.1858x over previous
"""Trainium2 Bass kernel for the 5-head detection tower (nn_DFD_10849087390476).

Network (per head h of 5): 1x1-conv tower on x [B,64,H,W]:
    h1 = relu(Win x + bin)
    h2 = h1 + relu(Wh0 h1 + bh0)
    h3 = h2 + relu(Wh1 h2 + bh1)
    out_h = Wout h3 + bout
Output = concat over heads: channels [cls 81, obj 2, box 4, pos 64, ins 128] = 279.

Sharding: data-parallel over (batch, H/2) -> 8 shards of 32768 pixels.
Per core pixels split in two 16384-px halves: A-px channels on SBUF partitions
0-63, B-px on 64-127 (2 pixels per column). Heads paired into three sections
with block-diagonal stationaries: co=cls+obj, bp=box+pos, ins=ins(A/B).

v2 layout: stage-major over groups of G=4 pair-tiles (512 cols each) so every
stationary streams 2048+ columns back-to-back (PE p-state stays at 2.4 GHz).
Activations in bf16: hidden residual adds run on Pool/DVE-2x from SBUF,
layer-2 residual is a fused scalar_tensor_tensor, in/l1 relus on the scalar
engine, out-proj copy (+bias via per-partition scalar AP) split scalar/DVE.
Output staged and stored as bf16 (halves write traffic), upcast on host.
"""
import numpy as np
import ml_dtypes

from concourse import bacc, tile
import concourse.mybir as mybir
from concourse.bass_utils import run_bass_kernel_spmd

F32 = mybir.dt.float32
F32R = mybir.dt.float32r
BF16 = mybir.dt.bfloat16
AF = mybir.ActivationFunctionType
ALU = mybir.AluOpType

B, C, H, W = 4, 64, 256, 256
NCORES = 8
NPX = (B * H * W) // NCORES          # 32768 pixels per core
NG = NPX // 2                        # 16384 per A/B half
T = 512                              # columns (pixel pairs) per matmul tile
GT = 4                               # pair-tiles per stage group
NGRP = NG // (GT * T)                # 8 groups per core
OD = 279

SECS = ("co", "bp", "ins")
MO = {"co": 83, "bp": 68, "ins": 128}
OCH = {"co": (0, 83), "bp": (83, 151), "ins": (151, 279)}

# ---- packed fp32 weight tensor (in-proj stationaries, bitcast f32r) ----
_WF = {}
_c = 0
for _n in ("sin_co", "sin_bp", "sin_ins"):
    _WF[_n] = (_c, 128); _c += 128
WF_COLS = _c

# ---- packed bf16 weight tensor (l1/l2/out stationaries) ----
_WB = {}
_c = 0
for _n in ("sl1_co", "sl1_bp", "sl1_ins", "sl2_co", "sl2_bp", "sl2_ins"):
    _WB[_n] = (_c, 128); _c += 128
for _s in SECS:
    _WB["sout_" + _s] = (_c, MO[_s] if _s != "ins" else 128); _c += _WB["sout_" + _s][1]
WB_COLS = _c

# ---- packed fp32 bias tensor (one column each) ----
_BI = {}
_c = 0
for _n in ("bin_co", "bin_bp", "bin_ins", "bh1_co", "bh1_bp", "bh1_ins",
           "bh2_co", "bh2_bp", "bh2_ins", "bout_co", "bout_bp", "bout_ins"):
    _BI[_n] = _c; _c += 1
BI_COLS = _c

import os
ADDS_ON_POOL = bool(int(os.environ.get("K_ADDS_ON_POOL", "0")))

_last_results = None                 # test.py reads exec_time_ns from here
_cache = {}


def _bd(a, b):
    out = np.zeros((a.shape[0] + b.shape[0], a.shape[1] + b.shape[1]), np.float32)
    out[:a.shape[0], :a.shape[1]] = a
    out[a.shape[0]:, a.shape[1]:] = b
    return out


def _build(fast: bool):
    nc = bacc.Bacc("TRN2", target_bir_lowering=False, debug=False)

    xs_d = nc.dram_tensor("xs", [128, NG], F32, kind="ExternalInput")
    wf_d = nc.dram_tensor("wf", [128, WF_COLS], F32, kind="ExternalInput")
    wb_d = nc.dram_tensor("wb", [128, WB_COLS], BF16, kind="ExternalInput")
    bi_d = nc.dram_tensor("bi", [128, BI_COLS], F32, kind="ExternalInput")
    out_d = nc.dram_tensor("out", [OD, NPX], BF16, kind="ExternalOutput")

    GC = GT * T                      # columns per group (2048)

    with tile.TileContext(nc) as tc:
        with tc.tile_pool(name="const", bufs=1) as cpool, \
             tc.tile_pool(name="xp", bufs=2) as xpool, \
             tc.tile_pool(name="hp", bufs=5) as hpool, \
             tc.tile_pool(name="op", bufs=2) as opool, \
             tc.tile_pool(name="psb", bufs=3, space="PSUM") as psb, \
             tc.tile_pool(name="pss", bufs=2, space="PSUM") as pss:

            wf_t = cpool.tile([128, WF_COLS], F32R, tag="wf")
            nc.sync.dma_start(out=wf_t[:], in_=wf_d.ap().bitcast(F32R))
            wb_t = cpool.tile([128, WB_COLS], BF16, tag="wb")
            nc.sync.dma_start(out=wb_t[:], in_=wb_d.ap())
            bi_t = cpool.tile([128, BI_COLS], F32, tag="bi")
            nc.sync.dma_start(out=bi_t[:], in_=bi_d.ap())

            def wfa(name):
                c0, n = _WF[name]
                return wf_t[:, c0:c0 + n]

            def wba(name):
                c0, n = _WB[name]
                return wb_t[:, c0:c0 + n]

            def bia(name, rows=128):
                c0 = _BI[name]
                return bi_t[0:rows, c0:c0 + 1]

            def load_x(g):
                x_t = xpool.tile([128, GC], F32R, tag="x", name=f"x_{g}")
                nc.sync.dma_start(
                    out=x_t[:],
                    in_=xs_d.ap()[:, g * GC:(g + 1) * GC].bitcast(F32R))
                return x_t

            x_next = load_x(0)
            for g in range(NGRP):
                x_t = x_next
                if g + 1 < NGRP:
                    x_next = load_x(g + 1)

                # output staging for this group: A and B halves per section
                oA = {s: opool.tile([MO[s], GC], BF16, tag=f"oA{s}", name=f"oA_{s}{g}")
                      for s in SECS}
                oB = {s: opool.tile([MO[s], GC], BF16, tag=f"oB{s}", name=f"oB_{s}{g}")
                      for s in SECS}

                for s in SECS:
                    wide = 2 * T if s != "ins" else T    # cols per pair-tile
                    pstag = "big" if s != "ins" else "small"
                    pspool = psb if s != "ins" else pss

                    # ---- in-proj ----
                    ps_in = []
                    for t in range(GT):
                        xg = x_t[:, t * T:(t + 1) * T]
                        p = pspool.tile([128, wide], F32, tag=pstag, name=f"pin_{s}{t}")
                        if s == "ins":
                            nc.tensor.matmul(p[:], wfa("sin_ins"), xg,
                                             start=True, stop=True)
                        else:
                            nc.tensor.matmul(p[:, 0:T], wfa("sin_" + s)[0:64, :],
                                             xg[0:64, :], start=True, stop=True)
                            nc.tensor.matmul(p[:, T:2 * T],
                                             wfa("sin_" + s)[64:128, :],
                                             xg[64:128, :], start=True, stop=True)
                        ps_in.append(p)
                    h1 = []
                    for t in range(GT):
                        ht = hpool.tile([128, wide], BF16, tag=f"h1{s}", name=f"h1_{s}{t}")
                        nc.scalar.activation(ht[:], ps_in[t][:], AF.Relu,
                                             bias=bia("bin_" + s), scale=1.0)
                        h1.append(ht)

                    # ---- hidden layer 1: r1 = relu(W1 h1 + bh1); h2 = h1 + r1
                    ps_l1 = []
                    for t in range(GT):
                        p = pspool.tile([128, wide], F32, tag=pstag, name=f"pl1_{s}{t}")
                        for k in range(wide // T):
                            nc.tensor.matmul(p[:, k * T:(k + 1) * T], wba(f"sl1_{s}"),
                                             h1[t][:, k * T:(k + 1) * T],
                                             start=True, stop=True)
                        ps_l1.append(p)
                    h2 = []
                    for t in range(GT):
                        rt = hpool.tile([128, wide], BF16, tag=f"r1{s}", name=f"r1_{s}{t}")
                        nc.scalar.activation(rt[:], ps_l1[t][:], AF.Relu,
                                             bias=bia("bh1_" + s), scale=1.0)
                        ht = hpool.tile([128, wide], BF16, tag=f"h2{s}", name=f"h2_{s}{t}")
                        # bf16 SBUF adds on DVE (2x_1p mode); ADDS_ON_POOL
                        # offloads co/bp adds to the GpSimd Q7s instead
                        if ADDS_ON_POOL and s != "ins":
                            nc.gpsimd.tensor_tensor(ht[:], h1[t][:], rt[:], ALU.add)
                        else:
                            nc.vector.tensor_tensor(ht[:], h1[t][:], rt[:], ALU.add)
                        h2.append(ht)

                    # ---- hidden layer 2 (fused residual on DVE) ----
                    ps_l2 = []
                    for t in range(GT):
                        p = pspool.tile([128, wide], F32, tag=pstag, name=f"pl2_{s}{t}")
                        for k in range(wide // T):
                            nc.tensor.matmul(p[:, k * T:(k + 1) * T], wba(f"sl2_{s}"),
                                             h2[t][:, k * T:(k + 1) * T],
                                             start=True, stop=True)
                        ps_l2.append(p)
                    h3 = []
                    for t in range(GT):
                        ht = hpool.tile([128, wide], BF16, tag=f"h3{s}", name=f"h3_{s}{t}")
                        if fast:
                            # h3 = max(psum, 0) + h2
                            nc.vector.scalar_tensor_tensor(
                                ht[:], ps_l2[t][:], 0.0, h2[t][:], ALU.max, ALU.add)
                        else:
                            rt = hpool.tile([128, wide], BF16, tag=f"r2{s}",
                                            name=f"r2_{s}{t}")
                            nc.scalar.activation(rt[:], ps_l2[t][:], AF.Relu,
                                                 bias=bia("bh2_" + s), scale=1.0)
                            nc.vector.tensor_tensor(ht[:], h2[t][:], rt[:], ALU.add)
                        h3.append(ht)

                    # ---- out-proj ----
                    mo = MO[s]
                    for t in range(GT):
                        p = psb.tile([128, 2 * T], F32, tag="big", name=f"po_{s}{t}")
                        if s == "ins":
                            # A-px into cols 0:T, B-px into T:2T
                            nc.tensor.matmul(p[:, 0:T], wba("sout_ins")[0:64, :],
                                             h3[t][0:64, :], start=True, stop=True)
                            nc.tensor.matmul(p[:, T:2 * T],
                                             wba("sout_ins")[64:128, :],
                                             h3[t][64:128, :], start=True, stop=True)
                        else:
                            nc.tensor.matmul(p[0:mo, 0:T], wba("sout_" + s),
                                             h3[t][:, 0:T], start=True, stop=True)
                            nc.tensor.matmul(p[0:mo, T:2 * T], wba("sout_" + s),
                                             h3[t][:, T:2 * T], start=True, stop=True)
                        # copy psum -> staging with bias; scalar takes co,
                        # DVE takes bp and ins
                        dA = oA[s][:, t * T:(t + 1) * T]
                        dB = oB[s][:, t * T:(t + 1) * T]
                        if s == "co":
                            nc.scalar.activation(dA, p[0:mo, 0:T], AF.Identity,
                                                 bias=bia("bout_co", rows=mo), scale=1.0)
                            nc.scalar.activation(dB, p[0:mo, T:2 * T], AF.Identity,
                                                 bias=bia("bout_co", rows=mo), scale=1.0)
                        else:
                            bap = bia("bout_" + s, rows=mo)
                            nc.vector.tensor_scalar(dA, p[0:mo, 0:T], bap, None, ALU.add)
                            nc.vector.tensor_scalar(dB, p[0:mo, T:2 * T], bap, None, ALU.add)

                # ---- flush group staging (pool engine: cheap DMA issue) ----
                base = g * GC
                for s in SECS:
                    lo, hi = OCH[s]
                    nc.gpsimd.dma_start(out=out_d.ap()[lo:hi, base:base + GC],
                                        in_=oA[s][:])
                    nc.gpsimd.dma_start(out=out_d.ap()[lo:hi, NG + base:NG + base + GC],
                                        in_=oB[s][:])

    nc.compile()
    return nc


def _prep_inputs(inputs):
    f32 = np.float32
    bf16 = ml_dtypes.bfloat16

    def wT(name):
        return np.ascontiguousarray(np.asarray(inputs[name], f32).T)

    # fp32 (f32r) in-proj stationaries: rows 0:64 serve A-px, 64:128 B-px
    wfm = {}
    wfm["sin_co"] = np.concatenate(
        [np.concatenate([wT("cls_Win"), wT("obj_Win")], 1)] * 2, 0)
    wfm["sin_bp"] = np.concatenate(
        [np.concatenate([wT("box_Win"), wT("pos_Win")], 1)] * 2, 0)
    wfm["sin_ins"] = _bd(wT("ins_Win"), wT("ins_Win"))
    wf = np.zeros((128, WF_COLS), f32)
    for name, (c0, n) in _WF.items():
        wf[:, c0:c0 + n] = wfm[name]

    # bf16 stationaries for l1/l2/out
    wbm = {}
    for l in (1, 2):
        wbm[f"sl{l}_co"] = _bd(np.asarray(inputs["cls_Wh"][l - 1], f32).T,
                               np.asarray(inputs["obj_Wh"][l - 1], f32).T)
        wbm[f"sl{l}_bp"] = _bd(np.asarray(inputs["box_Wh"][l - 1], f32).T,
                               np.asarray(inputs["pos_Wh"][l - 1], f32).T)
        wbm[f"sl{l}_ins"] = _bd(np.asarray(inputs["ins_Wh"][l - 1], f32).T,
                                np.asarray(inputs["ins_Wh"][l - 1], f32).T)
    wbm["sout_co"] = _bd(wT("cls_Wout"), wT("obj_Wout"))        # [128, 83]
    wbm["sout_bp"] = _bd(wT("box_Wout"), wT("pos_Wout"))        # [128, 68]
    wbm["sout_ins"] = np.concatenate([wT("ins_Wout")] * 2, 0)   # [128, 128]
    wb = np.zeros((128, WB_COLS), f32)
    for name, (c0, n) in _WB.items():
        wb[:, c0:c0 + n] = wbm[name]
    wb = wb.astype(bf16)

    def colv(v):
        return np.asarray(v, f32).reshape(-1)

    bim = {}
    bim["bin_co"] = np.concatenate([colv(inputs["cls_bin"]), colv(inputs["obj_bin"])])
    bim["bin_bp"] = np.concatenate([colv(inputs["box_bin"]), colv(inputs["pos_bin"])])
    bim["bin_ins"] = np.concatenate([colv(inputs["ins_bin"])] * 2)
    for l in (1, 2):
        bim[f"bh{l}_co"] = np.concatenate([colv(inputs["cls_bh"][l - 1]),
                                           colv(inputs["obj_bh"][l - 1])])
        bim[f"bh{l}_bp"] = np.concatenate([colv(inputs["box_bh"][l - 1]),
                                           colv(inputs["pos_bh"][l - 1])])
        bim[f"bh{l}_ins"] = np.concatenate([colv(inputs["ins_bh"][l - 1])] * 2)
    bim["bout_co"] = np.concatenate([colv(inputs["cls_bout"]), colv(inputs["obj_bout"])])
    bim["bout_bp"] = np.concatenate([colv(inputs["box_bout"]), colv(inputs["pos_bout"])])
    bim["bout_ins"] = colv(inputs["ins_bout"])
    bi = np.zeros((128, BI_COLS), f32)
    for name, c0 in _BI.items():
        v = bim[name]
        bi[:v.shape[0], c0] = v

    # fast path requires zero layer-2 hidden biases (fused STT cannot apply a
    # bias before the relu)
    fast = not any(np.any(np.asarray(inputs[k + "_bh"][1])) for k in
                   ("cls", "obj", "box", "pos", "ins"))

    x = np.asarray(inputs["x"], f32)
    in_maps = []
    for c in range(NCORES):
        b, hh = c // 2, c % 2
        xs = x[b, :, hh * 128:(hh + 1) * 128, :].reshape(64, NPX)
        xsr = np.ascontiguousarray(
            np.concatenate([xs[:, :NG], xs[:, NG:]], axis=0))   # [128, NG]
        in_maps.append({"xs": xsr, "wf": wf, "wb": wb, "bi": bi})
    return in_maps, fast


def kernel(**inputs) -> np.ndarray:
    global _last_results
    in_maps, fast = _prep_inputs(inputs)
    if fast not in _cache:
        _cache[fast] = _build(fast)
    nc = _cache[fast]
    res = run_bass_kernel_spmd(nc, in_maps, core_ids=list(range(NCORES)))
    _last_results = res

    out = np.empty((B, OD, H, W), np.float32)
    for c in range(NCORES):
        b, hh = c // 2, c % 2
        o = np.asarray(res.results[c]["out"])
        if o.dtype != np.float32:
            o = o.astype(np.float32)
        out[b, :, hh * 128:(hh + 1) * 128, :] = o.reshape(OD, 128, W)
    return out


# revision 11
# speedup vs baseline: 1.2038x; 1.0151x over previous
"""Trainium2 Bass kernel for the 5-head detection tower (nn_DFD_10849087390476).

Network (per head h of 5): 1x1-conv tower on x [B,64,H,W]:
    h1 = relu(Win x + bin)
    h2 = h1 + relu(Wh0 h1 + bh0)
    h3 = h2 + relu(Wh1 h2 + bh1)
    out_h = Wout h3 + bout
Output = concat over heads: channels [cls 81, obj 2, box 4, pos 64, ins 128] = 279.

Sharding: data-parallel over (batch, H/2) -> 8 shards of 32768 pixels.
Per core pixels split in two 16384-px halves: A-px channels on SBUF partitions
0-63, B-px on 64-127 (2 pixels per column). Heads paired into three sections
with block-diagonal stationaries: co=cls+obj, bp=box+pos, ins=ins(A/B).

v2 layout: stage-major over groups of G=4 pair-tiles (512 cols each) so every
stationary streams 2048+ columns back-to-back (PE p-state stays at 2.4 GHz).
Activations in bf16: hidden residual adds run on Pool/DVE-2x from SBUF,
layer-2 residual is a fused scalar_tensor_tensor, in/l1 relus on the scalar
engine, out-proj copy (+bias via per-partition scalar AP) split scalar/DVE.
Output staged and stored as bf16 (halves write traffic), upcast on host.
"""
import numpy as np
import ml_dtypes

from concourse import bacc, tile
import concourse.mybir as mybir
from concourse.bass_utils import run_bass_kernel_spmd

F32 = mybir.dt.float32
F32R = mybir.dt.float32r
BF16 = mybir.dt.bfloat16
AF = mybir.ActivationFunctionType
ALU = mybir.AluOpType

B, C, H, W = 4, 64, 256, 256
NCORES = 8
NPX = (B * H * W) // NCORES          # 32768 pixels per core
NG = NPX // 2                        # 16384 per A/B half
T = 512                              # columns (pixel pairs) per matmul tile
GT = 4                               # pair-tiles per stage group
NGRP = NG // (GT * T)                # 8 groups per core
OD = 279

SECS = ("co", "bp", "ins")
MO = {"co": 83, "bp": 68, "ins": 128}
OCH = {"co": (0, 83), "bp": (83, 151), "ins": (151, 279)}

# ---- packed bf16 weight tensor (all stationaries) ----
_WB = {}
_c = 0
for _n in ("sin_co", "sin_bp", "sin_ins",
           "sl1_co", "sl1_bp", "sl1_ins", "sl2_co", "sl2_bp", "sl2_ins"):
    _WB[_n] = (_c, 128); _c += 128
for _s in SECS:
    _WB["sout_" + _s] = (_c, MO[_s] if _s != "ins" else 128); _c += _WB["sout_" + _s][1]
WB_COLS = _c

# ---- packed fp32 bias tensor (one column each) ----
_BI = {}
_c = 0
for _n in ("bin_co", "bin_bp", "bin_ins", "bh1_co", "bh1_bp", "bh1_ins",
           "bh2_co", "bh2_bp", "bh2_ins", "bout_co", "bout_bp", "bout_ins"):
    _BI[_n] = _c; _c += 1
BI_COLS = _c

import os
ADDS_ON_POOL = bool(int(os.environ.get("K_ADDS_ON_POOL", "0")))

_last_results = None                 # test.py reads exec_time_ns from here
_cache = {}


def _bd(a, b):
    out = np.zeros((a.shape[0] + b.shape[0], a.shape[1] + b.shape[1]), np.float32)
    out[:a.shape[0], :a.shape[1]] = a
    out[a.shape[0]:, a.shape[1]:] = b
    return out


def _build(fast: bool):
    nc = bacc.Bacc("TRN2", target_bir_lowering=False, debug=False)

    xs_d = nc.dram_tensor("xs", [128, NG], BF16, kind="ExternalInput")
    wb_d = nc.dram_tensor("wb", [128, WB_COLS], BF16, kind="ExternalInput")
    bi_d = nc.dram_tensor("bi", [128, BI_COLS], F32, kind="ExternalInput")
    out_d = nc.dram_tensor("out", [OD, NPX], BF16, kind="ExternalOutput")

    GC = GT * T                      # columns per group (2048)

    with tile.TileContext(nc) as tc:
        with tc.tile_pool(name="const", bufs=1) as cpool, \
             tc.tile_pool(name="xp", bufs=2) as xpool, \
             tc.tile_pool(name="hp", bufs=5) as hpool, \
             tc.tile_pool(name="op", bufs=2) as opool, \
             tc.tile_pool(name="psb", bufs=3, space="PSUM") as psb, \
             tc.tile_pool(name="pss", bufs=2, space="PSUM") as pss:

            wb_t = cpool.tile([128, WB_COLS], BF16, tag="wb")
            nc.sync.dma_start(out=wb_t[:], in_=wb_d.ap())
            bi_t = cpool.tile([128, BI_COLS], F32, tag="bi")
            nc.sync.dma_start(out=bi_t[:], in_=bi_d.ap())

            def wba(name):
                c0, n = _WB[name]
                return wb_t[:, c0:c0 + n]

            def bia(name, rows=128):
                c0 = _BI[name]
                return bi_t[0:rows, c0:c0 + 1]

            def load_x(g):
                x_t = xpool.tile([128, GC], BF16, tag="x", name=f"x_{g}")
                nc.sync.dma_start(
                    out=x_t[:],
                    in_=xs_d.ap()[:, g * GC:(g + 1) * GC])
                return x_t

            x_next = load_x(0)
            for g in range(NGRP):
                x_t = x_next
                if g + 1 < NGRP:
                    x_next = load_x(g + 1)

                # output staging for this group: A and B halves per section
                oA = {s: opool.tile([MO[s], GC], BF16, tag=f"oA{s}", name=f"oA_{s}{g}")
                      for s in SECS}
                oB = {s: opool.tile([MO[s], GC], BF16, tag=f"oB{s}", name=f"oB_{s}{g}")
                      for s in SECS}

                for s in SECS:
                    wide = 2 * T if s != "ins" else T    # cols per pair-tile
                    pstag = "big" if s != "ins" else "small"
                    pspool = psb if s != "ins" else pss

                    # ---- in-proj ----
                    ps_in = []
                    for t in range(GT):
                        xg = x_t[:, t * T:(t + 1) * T]
                        p = pspool.tile([128, wide], F32, tag=pstag, name=f"pin_{s}{t}")
                        if s == "ins":
                            nc.tensor.matmul(p[:], wba("sin_ins"), xg,
                                             start=True, stop=True)
                        else:
                            nc.tensor.matmul(p[:, 0:T], wba("sin_" + s)[0:64, :],
                                             xg[0:64, :], start=True, stop=True)
                            nc.tensor.matmul(p[:, T:2 * T],
                                             wba("sin_" + s)[64:128, :],
                                             xg[64:128, :], start=True, stop=True)
                        ps_in.append(p)
                    h1 = []
                    for t in range(GT):
                        ht = hpool.tile([128, wide], BF16, tag=f"h1{s}", name=f"h1_{s}{t}")
                        nc.scalar.activation(ht[:], ps_in[t][:], AF.Relu,
                                             bias=bia("bin_" + s), scale=1.0)
                        h1.append(ht)

                    # ---- hidden layer 1: r1 = relu(W1 h1 + bh1); h2 = h1 + r1
                    ps_l1 = []
                    for t in range(GT):
                        p = pspool.tile([128, wide], F32, tag=pstag, name=f"pl1_{s}{t}")
                        for k in range(wide // T):
                            nc.tensor.matmul(p[:, k * T:(k + 1) * T], wba(f"sl1_{s}"),
                                             h1[t][:, k * T:(k + 1) * T],
                                             start=True, stop=True)
                        ps_l1.append(p)
                    h2 = []
                    for t in range(GT):
                        rt = hpool.tile([128, wide], BF16, tag=f"r1{s}", name=f"r1_{s}{t}")
                        nc.scalar.activation(rt[:], ps_l1[t][:], AF.Relu,
                                             bias=bia("bh1_" + s), scale=1.0)
                        ht = hpool.tile([128, wide], BF16, tag=f"h2{s}", name=f"h2_{s}{t}")
                        # bf16 SBUF adds on DVE (2x_1p mode); ADDS_ON_POOL
                        # offloads co/bp adds to the GpSimd Q7s instead
                        if ADDS_ON_POOL and s != "ins":
                            nc.gpsimd.tensor_tensor(ht[:], h1[t][:], rt[:], ALU.add)
                        else:
                            nc.vector.tensor_tensor(ht[:], h1[t][:], rt[:], ALU.add)
                        h2.append(ht)

                    # ---- hidden layer 2 (fused residual on DVE) ----
                    ps_l2 = []
                    for t in range(GT):
                        p = pspool.tile([128, wide], F32, tag=pstag, name=f"pl2_{s}{t}")
                        for k in range(wide // T):
                            nc.tensor.matmul(p[:, k * T:(k + 1) * T], wba(f"sl2_{s}"),
                                             h2[t][:, k * T:(k + 1) * T],
                                             start=True, stop=True)
                        ps_l2.append(p)
                    h3 = []
                    for t in range(GT):
                        ht = hpool.tile([128, wide], BF16, tag=f"h3{s}", name=f"h3_{s}{t}")
                        if fast:
                            # h3 = max(psum, 0) + h2
                            nc.vector.scalar_tensor_tensor(
                                ht[:], ps_l2[t][:], 0.0, h2[t][:], ALU.max, ALU.add)
                        else:
                            rt = hpool.tile([128, wide], BF16, tag=f"r2{s}",
                                            name=f"r2_{s}{t}")
                            nc.scalar.activation(rt[:], ps_l2[t][:], AF.Relu,
                                                 bias=bia("bh2_" + s), scale=1.0)
                            nc.vector.tensor_tensor(ht[:], h2[t][:], rt[:], ALU.add)
                        h3.append(ht)

                    # ---- out-proj ----
                    mo = MO[s]
                    for t in range(GT):
                        p = psb.tile([128, 2 * T], F32, tag="big", name=f"po_{s}{t}")
                        if s == "ins":
                            # A-px into cols 0:T, B-px into T:2T
                            nc.tensor.matmul(p[:, 0:T], wba("sout_ins")[0:64, :],
                                             h3[t][0:64, :], start=True, stop=True)
                            nc.tensor.matmul(p[:, T:2 * T],
                                             wba("sout_ins")[64:128, :],
                                             h3[t][64:128, :], start=True, stop=True)
                        else:
                            nc.tensor.matmul(p[0:mo, 0:T], wba("sout_" + s),
                                             h3[t][:, 0:T], start=True, stop=True)
                            nc.tensor.matmul(p[0:mo, T:2 * T], wba("sout_" + s),
                                             h3[t][:, T:2 * T], start=True, stop=True)
                        # copy psum -> staging with bias; scalar takes co,
                        # DVE takes bp and ins
                        dA = oA[s][:, t * T:(t + 1) * T]
                        dB = oB[s][:, t * T:(t + 1) * T]
                        if s == "co":
                            nc.scalar.activation(dA, p[0:mo, 0:T], AF.Identity,
                                                 bias=bia("bout_co", rows=mo), scale=1.0)
                            nc.scalar.activation(dB, p[0:mo, T:2 * T], AF.Identity,
                                                 bias=bia("bout_co", rows=mo), scale=1.0)
                        else:
                            bap = bia("bout_" + s, rows=mo)
                            nc.vector.tensor_scalar(dA, p[0:mo, 0:T], bap, None, ALU.add)
                            nc.vector.tensor_scalar(dB, p[0:mo, T:2 * T], bap, None, ALU.add)

                # ---- flush group staging (pool engine: cheap DMA issue) ----
                base = g * GC
                for s in SECS:
                    lo, hi = OCH[s]
                    nc.gpsimd.dma_start(out=out_d.ap()[lo:hi, base:base + GC],
                                        in_=oA[s][:])
                    nc.gpsimd.dma_start(out=out_d.ap()[lo:hi, NG + base:NG + base + GC],
                                        in_=oB[s][:])

    nc.compile()
    return nc


def _prep_inputs(inputs):
    f32 = np.float32
    bf16 = ml_dtypes.bfloat16

    def wT(name):
        return np.ascontiguousarray(np.asarray(inputs[name], f32).T)

    # bf16 stationaries; in-proj rows 0:64 serve A-px, 64:128 B-px
    wbm = {}
    wbm["sin_co"] = np.concatenate(
        [np.concatenate([wT("cls_Win"), wT("obj_Win")], 1)] * 2, 0)
    wbm["sin_bp"] = np.concatenate(
        [np.concatenate([wT("box_Win"), wT("pos_Win")], 1)] * 2, 0)
    wbm["sin_ins"] = _bd(wT("ins_Win"), wT("ins_Win"))
    for l in (1, 2):
        wbm[f"sl{l}_co"] = _bd(np.asarray(inputs["cls_Wh"][l - 1], f32).T,
                               np.asarray(inputs["obj_Wh"][l - 1], f32).T)
        wbm[f"sl{l}_bp"] = _bd(np.asarray(inputs["box_Wh"][l - 1], f32).T,
                               np.asarray(inputs["pos_Wh"][l - 1], f32).T)
        wbm[f"sl{l}_ins"] = _bd(np.asarray(inputs["ins_Wh"][l - 1], f32).T,
                                np.asarray(inputs["ins_Wh"][l - 1], f32).T)
    wbm["sout_co"] = _bd(wT("cls_Wout"), wT("obj_Wout"))        # [128, 83]
    wbm["sout_bp"] = _bd(wT("box_Wout"), wT("pos_Wout"))        # [128, 68]
    wbm["sout_ins"] = np.concatenate([wT("ins_Wout")] * 2, 0)   # [128, 128]
    wb = np.zeros((128, WB_COLS), f32)
    for name, (c0, n) in _WB.items():
        wb[:, c0:c0 + n] = wbm[name]
    wb = wb.astype(bf16)

    def colv(v):
        return np.asarray(v, f32).reshape(-1)

    bim = {}
    bim["bin_co"] = np.concatenate([colv(inputs["cls_bin"]), colv(inputs["obj_bin"])])
    bim["bin_bp"] = np.concatenate([colv(inputs["box_bin"]), colv(inputs["pos_bin"])])
    bim["bin_ins"] = np.concatenate([colv(inputs["ins_bin"])] * 2)
    for l in (1, 2):
        bim[f"bh{l}_co"] = np.concatenate([colv(inputs["cls_bh"][l - 1]),
                                           colv(inputs["obj_bh"][l - 1])])
        bim[f"bh{l}_bp"] = np.concatenate([colv(inputs["box_bh"][l - 1]),
                                           colv(inputs["pos_bh"][l - 1])])
        bim[f"bh{l}_ins"] = np.concatenate([colv(inputs["ins_bh"][l - 1])] * 2)
    bim["bout_co"] = np.concatenate([colv(inputs["cls_bout"]), colv(inputs["obj_bout"])])
    bim["bout_bp"] = np.concatenate([colv(inputs["box_bout"]), colv(inputs["pos_bout"])])
    bim["bout_ins"] = colv(inputs["ins_bout"])
    bi = np.zeros((128, BI_COLS), f32)
    for name, c0 in _BI.items():
        v = bim[name]
        bi[:v.shape[0], c0] = v

    # fast path requires zero layer-2 hidden biases (fused STT cannot apply a
    # bias before the relu)
    fast = not any(np.any(np.asarray(inputs[k + "_bh"][1])) for k in
                   ("cls", "obj", "box", "pos", "ins"))

    x = np.asarray(inputs["x"], f32)
    in_maps = []
    for c in range(NCORES):
        b, hh = c // 2, c % 2
        xs = x[b, :, hh * 128:(hh + 1) * 128, :].reshape(64, NPX)
        xsr = np.ascontiguousarray(
            np.concatenate([xs[:, :NG], xs[:, NG:]], axis=0)).astype(bf16)
        in_maps.append({"xs": xsr, "wb": wb, "bi": bi})
    return in_maps, fast


def kernel(**inputs) -> np.ndarray:
    global _last_results
    in_maps, fast = _prep_inputs(inputs)
    if fast not in _cache:
        _cache[fast] = _build(fast)
    nc = _cache[fast]
    res = run_bass_kernel_spmd(nc, in_maps, core_ids=list(range(NCORES)))
    _last_results = res

    out = np.empty((B, OD, H, W), np.float32)
    for c in range(NCORES):
        b, hh = c // 2, c % 2
        o = np.asarray(res.results[c]["out"])
        if o.dtype != np.float32:
            o = o.astype(np.float32)
        out[b, :, hh * 128:(hh + 1) * 128, :] = o.reshape(OD, 128, W)
    return out


# revision 12
# speedup vs baseline: 1.3765x; 1.1435x over previous
"""Trainium2 Bass kernel for the 5-head detection tower (nn_DFD_10849087390476).

Network (per head h of 5): 1x1-conv tower on x [B,64,H,W]:
    h1 = relu(Win x + bin)
    h2 = h1 + relu(Wh0 h1 + bh0)
    h3 = h2 + relu(Wh1 h2 + bh1)
    out_h = Wout h3 + bout
Output = concat over heads: channels [cls 81, obj 2, box 4, pos 64, ins 128] = 279.

Sharding: data-parallel over (batch, H/2) -> 8 shards of 32768 pixels.
Per core pixels split in two 16384-px halves: A-px channels on SBUF partitions
0-63, B-px on 64-127 (2 pixels per column). Heads paired into three sections
with block-diagonal stationaries: co=cls+obj, bp=box+pos, ins=ins(A/B).

v2 layout: stage-major over groups of G=4 pair-tiles (512 cols each) so every
stationary streams 2048+ columns back-to-back (PE p-state stays at 2.4 GHz).
Activations in bf16: hidden residual adds run on Pool/DVE-2x from SBUF,
layer-2 residual is a fused scalar_tensor_tensor, in/l1 relus on the scalar
engine, out-proj copy (+bias via per-partition scalar AP) split scalar/DVE.
Output staged and stored as bf16 (halves write traffic), upcast on host.
"""
import numpy as np
import ml_dtypes

from concourse import bacc, tile
import concourse.mybir as mybir
from concourse.bass_utils import run_bass_kernel_spmd

F32 = mybir.dt.float32
F32R = mybir.dt.float32r
BF16 = mybir.dt.bfloat16
AF = mybir.ActivationFunctionType
ALU = mybir.AluOpType

B, C, H, W = 4, 64, 256, 256
NCORES = 8
NPX = (B * H * W) // NCORES          # 32768 pixels per core
NG = NPX // 2                        # 16384 per A/B half
T = 512                              # columns (pixel pairs) per matmul tile
GT = 4                               # pair-tiles per stage group
NGRP = NG // (GT * T)                # 8 groups per core
OD = 279

SECS = ("co", "bp", "ins")
MO = {"co": 83, "bp": 68, "ins": 128}
OCH = {"co": (0, 83), "bp": (83, 151), "ins": (151, 279)}

# ---- packed bf16 weight tensor (all stationaries) ----
_WB = {}
_c = 0
for _n in ("sin_co", "sin_bp", "sin_ins",
           "sl1_co", "sl1_bp", "sl1_ins", "sl2_co", "sl2_bp", "sl2_ins"):
    _WB[_n] = (_c, 128); _c += 128
for _s in SECS:
    _WB["sout_" + _s] = (_c, MO[_s] if _s != "ins" else 128); _c += _WB["sout_" + _s][1]
WB_COLS = _c

# ---- packed fp32 bias tensor (one column each) ----
_BI = {}
_c = 0
for _n in ("bin_co", "bin_bp", "bin_ins", "bh1_co", "bh1_bp", "bh1_ins",
           "bh2_co", "bh2_bp", "bh2_ins", "bout_co", "bout_bp", "bout_ins"):
    _BI[_n] = _c; _c += 1
BI_COLS = _c

import os
ADDS_ON_POOL = bool(int(os.environ.get("K_ADDS_ON_POOL", "0")))

_last_results = None                 # test.py reads exec_time_ns from here
_cache = {}


def _bd(a, b):
    out = np.zeros((a.shape[0] + b.shape[0], a.shape[1] + b.shape[1]), np.float32)
    out[:a.shape[0], :a.shape[1]] = a
    out[a.shape[0]:, a.shape[1]:] = b
    return out


def _build(fast: bool):
    nc = bacc.Bacc("TRN2", target_bir_lowering=False, debug=False)

    xs_d = nc.dram_tensor("xs", [128, NG], BF16, kind="ExternalInput")
    wb_d = nc.dram_tensor("wb", [128, WB_COLS], BF16, kind="ExternalInput")
    bi_d = nc.dram_tensor("bi", [128, BI_COLS], F32, kind="ExternalInput")
    out_d = nc.dram_tensor("out", [OD, NPX], BF16, kind="ExternalOutput")

    GC = GT * T                      # columns per group (2048)

    with tile.TileContext(nc) as tc:
        with tc.tile_pool(name="const", bufs=1) as cpool, \
             tc.tile_pool(name="xp", bufs=2) as xpool, \
             tc.tile_pool(name="hp", bufs=5) as hpool, \
             tc.tile_pool(name="op", bufs=2) as opool, \
             tc.tile_pool(name="psb", bufs=3, space="PSUM") as psb, \
             tc.tile_pool(name="pss", bufs=2, space="PSUM") as pss:

            wb_t = cpool.tile([128, WB_COLS], BF16, tag="wb")
            nc.sync.dma_start(out=wb_t[:], in_=wb_d.ap())
            bi_t = cpool.tile([128, BI_COLS], F32, tag="bi")
            nc.sync.dma_start(out=bi_t[:], in_=bi_d.ap())

            def wba(name):
                c0, n = _WB[name]
                return wb_t[:, c0:c0 + n]

            def bia(name, rows=128):
                c0 = _BI[name]
                return bi_t[0:rows, c0:c0 + 1]

            def load_x(g):
                x_t = xpool.tile([128, GC], BF16, tag="x", name=f"x_{g}")
                nc.sync.dma_start(
                    out=x_t[:],
                    in_=xs_d.ap()[:, g * GC:(g + 1) * GC])
                return x_t

            x_next = load_x(0)
            for g in range(NGRP):
                x_t = x_next
                if g + 1 < NGRP:
                    x_next = load_x(g + 1)

                # output staging for this group: [mo, A-cols | B-cols]
                ost = {s: opool.tile([MO[s], 2 * GC], BF16, tag=f"o{s}",
                                     name=f"o_{s}{g}")
                       for s in SECS}

                for s in SECS:
                    wide = 2 * T if s != "ins" else T    # cols per pair-tile
                    pstag = "big" if s != "ins" else "small"
                    pspool = psb if s != "ins" else pss

                    # ---- in-proj ----
                    ps_in = []
                    for t in range(GT):
                        xg = x_t[:, t * T:(t + 1) * T]
                        p = pspool.tile([128, wide], F32, tag=pstag, name=f"pin_{s}{t}")
                        if s == "ins":
                            nc.tensor.matmul(p[:], wba("sin_ins"), xg,
                                             start=True, stop=True)
                        else:
                            nc.tensor.matmul(p[:, 0:T], wba("sin_" + s)[0:64, :],
                                             xg[0:64, :], start=True, stop=True)
                            nc.tensor.matmul(p[:, T:2 * T],
                                             wba("sin_" + s)[64:128, :],
                                             xg[64:128, :], start=True, stop=True)
                        ps_in.append(p)
                    h1 = []
                    for t in range(GT):
                        ht = hpool.tile([128, wide], BF16, tag=f"h1{s}", name=f"h1_{s}{t}")
                        nc.scalar.activation(ht[:], ps_in[t][:], AF.Relu,
                                             bias=bia("bin_" + s), scale=1.0)
                        h1.append(ht)

                    # ---- hidden layer 1: r1 = relu(W1 h1 + bh1); h2 = h1 + r1
                    ps_l1 = []
                    for t in range(GT):
                        p = pspool.tile([128, wide], F32, tag=pstag, name=f"pl1_{s}{t}")
                        for k in range(wide // T):
                            nc.tensor.matmul(p[:, k * T:(k + 1) * T], wba(f"sl1_{s}"),
                                             h1[t][:, k * T:(k + 1) * T],
                                             start=True, stop=True)
                        ps_l1.append(p)
                    h2 = []
                    for t in range(GT):
                        ht = hpool.tile([128, wide], BF16, tag=f"h2{s}", name=f"h2_{s}{t}")
                        if fast:
                            # h2 = max(psum, 0) + h1, fused on DVE
                            nc.vector.scalar_tensor_tensor(
                                ht[:], ps_l1[t][:], 0.0, h1[t][:], ALU.max, ALU.add)
                        else:
                            rt = hpool.tile([128, wide], BF16, tag=f"r1{s}",
                                            name=f"r1_{s}{t}")
                            nc.scalar.activation(rt[:], ps_l1[t][:], AF.Relu,
                                                 bias=bia("bh1_" + s), scale=1.0)
                            nc.vector.tensor_tensor(ht[:], h1[t][:], rt[:], ALU.add)
                        h2.append(ht)

                    # ---- hidden layer 2 (fused residual on DVE) ----
                    ps_l2 = []
                    for t in range(GT):
                        p = pspool.tile([128, wide], F32, tag=pstag, name=f"pl2_{s}{t}")
                        for k in range(wide // T):
                            nc.tensor.matmul(p[:, k * T:(k + 1) * T], wba(f"sl2_{s}"),
                                             h2[t][:, k * T:(k + 1) * T],
                                             start=True, stop=True)
                        ps_l2.append(p)
                    h3 = []
                    for t in range(GT):
                        ht = hpool.tile([128, wide], BF16, tag=f"h3{s}", name=f"h3_{s}{t}")
                        if fast:
                            # h3 = max(psum, 0) + h2
                            nc.vector.scalar_tensor_tensor(
                                ht[:], ps_l2[t][:], 0.0, h2[t][:], ALU.max, ALU.add)
                        else:
                            rt = hpool.tile([128, wide], BF16, tag=f"r2{s}",
                                            name=f"r2_{s}{t}")
                            nc.scalar.activation(rt[:], ps_l2[t][:], AF.Relu,
                                                 bias=bia("bh2_" + s), scale=1.0)
                            nc.vector.tensor_tensor(ht[:], h2[t][:], rt[:], ALU.add)
                        h3.append(ht)

                    # ---- out-proj ----
                    mo = MO[s]
                    for t in range(GT):
                        p = psb.tile([128, 2 * T], F32, tag="big", name=f"po_{s}{t}")
                        if s == "ins":
                            # A-px into cols 0:T, B-px into T:2T
                            nc.tensor.matmul(p[:, 0:T], wba("sout_ins")[0:64, :],
                                             h3[t][0:64, :], start=True, stop=True)
                            nc.tensor.matmul(p[:, T:2 * T],
                                             wba("sout_ins")[64:128, :],
                                             h3[t][64:128, :], start=True, stop=True)
                        else:
                            nc.tensor.matmul(p[0:mo, 0:T], wba("sout_" + s),
                                             h3[t][:, 0:T], start=True, stop=True)
                            nc.tensor.matmul(p[0:mo, T:2 * T], wba("sout_" + s),
                                             h3[t][:, T:2 * T], start=True, stop=True)
                        # copy psum -> staging with bias on the scalar engine.
                        # psum cols are [A 512 | B 512]; dst is a 3D AP putting
                        # A at col t*T and B at col GC + t*T of the staging tile
                        dst = ost[s][:].rearrange("p (i c) -> p i c", i=2)[
                            :, :, t * T:(t + 1) * T]
                        nc.scalar.activation(dst, p[0:mo, :], AF.Identity,
                                             bias=bia("bout_" + s, rows=mo), scale=1.0)

                # ---- flush group staging (pool engine: cheap DMA issue) ----
                base = g * GC
                for s in SECS:
                    lo, hi = OCH[s]
                    nc.gpsimd.dma_start(out=out_d.ap()[lo:hi, base:base + GC],
                                        in_=ost[s][:, 0:GC])
                    nc.gpsimd.dma_start(out=out_d.ap()[lo:hi, NG + base:NG + base + GC],
                                        in_=ost[s][:, GC:2 * GC])

    nc.compile()
    return nc


def _prep_inputs(inputs):
    f32 = np.float32
    bf16 = ml_dtypes.bfloat16

    def wT(name):
        return np.ascontiguousarray(np.asarray(inputs[name], f32).T)

    # bf16 stationaries; in-proj rows 0:64 serve A-px, 64:128 B-px
    wbm = {}
    wbm["sin_co"] = np.concatenate(
        [np.concatenate([wT("cls_Win"), wT("obj_Win")], 1)] * 2, 0)
    wbm["sin_bp"] = np.concatenate(
        [np.concatenate([wT("box_Win"), wT("pos_Win")], 1)] * 2, 0)
    wbm["sin_ins"] = _bd(wT("ins_Win"), wT("ins_Win"))
    for l in (1, 2):
        wbm[f"sl{l}_co"] = _bd(np.asarray(inputs["cls_Wh"][l - 1], f32).T,
                               np.asarray(inputs["obj_Wh"][l - 1], f32).T)
        wbm[f"sl{l}_bp"] = _bd(np.asarray(inputs["box_Wh"][l - 1], f32).T,
                               np.asarray(inputs["pos_Wh"][l - 1], f32).T)
        wbm[f"sl{l}_ins"] = _bd(np.asarray(inputs["ins_Wh"][l - 1], f32).T,
                                np.asarray(inputs["ins_Wh"][l - 1], f32).T)
    wbm["sout_co"] = _bd(wT("cls_Wout"), wT("obj_Wout"))        # [128, 83]
    wbm["sout_bp"] = _bd(wT("box_Wout"), wT("pos_Wout"))        # [128, 68]
    wbm["sout_ins"] = np.concatenate([wT("ins_Wout")] * 2, 0)   # [128, 128]
    wb = np.zeros((128, WB_COLS), f32)
    for name, (c0, n) in _WB.items():
        wb[:, c0:c0 + n] = wbm[name]
    wb = wb.astype(bf16)

    def colv(v):
        return np.asarray(v, f32).reshape(-1)

    bim = {}
    bim["bin_co"] = np.concatenate([colv(inputs["cls_bin"]), colv(inputs["obj_bin"])])
    bim["bin_bp"] = np.concatenate([colv(inputs["box_bin"]), colv(inputs["pos_bin"])])
    bim["bin_ins"] = np.concatenate([colv(inputs["ins_bin"])] * 2)
    for l in (1, 2):
        bim[f"bh{l}_co"] = np.concatenate([colv(inputs["cls_bh"][l - 1]),
                                           colv(inputs["obj_bh"][l - 1])])
        bim[f"bh{l}_bp"] = np.concatenate([colv(inputs["box_bh"][l - 1]),
                                           colv(inputs["pos_bh"][l - 1])])
        bim[f"bh{l}_ins"] = np.concatenate([colv(inputs["ins_bh"][l - 1])] * 2)
    bim["bout_co"] = np.concatenate([colv(inputs["cls_bout"]), colv(inputs["obj_bout"])])
    bim["bout_bp"] = np.concatenate([colv(inputs["box_bout"]), colv(inputs["pos_bout"])])
    bim["bout_ins"] = colv(inputs["ins_bout"])
    bi = np.zeros((128, BI_COLS), f32)
    for name, c0 in _BI.items():
        v = bim[name]
        bi[:v.shape[0], c0] = v

    # fast path requires zero layer-2 hidden biases (fused STT cannot apply a
    # bias before the relu)
    fast = not any(np.any(np.asarray(inputs[k + "_bh"][1])) for k in
                   ("cls", "obj", "box", "pos", "ins"))

    x = np.asarray(inputs["x"], f32)
    in_maps = []
    for c in range(NCORES):
        b, hh = c // 2, c % 2
        xs = x[b, :, hh * 128:(hh + 1) * 128, :].reshape(64, NPX)
        xsr = np.ascontiguousarray(
            np.concatenate([xs[:, :NG], xs[:, NG:]], axis=0)).astype(bf16)
        in_maps.append({"xs": xsr, "wb": wb, "bi": bi})
    return in_maps, fast


def kernel(**inputs) -> np.ndarray:
    global _last_results
    in_maps, fast = _prep_inputs(inputs)
    if fast not in _cache:
        _cache[fast] = _build(fast)
    nc = _cache[fast]
    res = run_bass_kernel_spmd(nc, in_maps, core_ids=list(range(NCORES)))
    _last_results = res

    out = np.empty((B, OD, H, W), np.float32)
    for c in range(NCORES):
        b, hh = c // 2, c % 2
        o = np.asarray(res.results[c]["out"])
        if o.dtype != np.float32:
            o = o.astype(np.float32)
        out[b, :, hh * 128:(hh + 1) * 128, :] = o.reshape(OD, 128, W)
    return out
